# revision 1
# baseline (speedup 1.0000x reference)
"""Trainium2 Bass kernel for a 12-layer BERT generator model.

Model (see problem reference):
  B=8, S=512, H=768, L=12, NH=12 (DH=64), FF=3072, V=21128
  - embedding gather + pos/type embeddings + LN
  - L x { QA-causal masked multi-head attention + LN, exact-GELU FFN + LN }
  - vocab projection [S, V]

Sharding: data-parallel, one sample per NeuronCore (B == n_cores == 8).
Each core runs an identical program; in_maps carry the per-core sample ids
plus (replicated) weights.

Per-core layout conventions:
  - activations are FEATURE-major in SBUF: hT[p, kf, s] = h[s, kf*128+p]
    (shape [128, H//128, S]) so that every dense layer is
        out[f_out, s] = sum_f W[f, f_out] * hT[f, s]
    i.e. matmul(lhsT=W-slice [128, 128], rhs=hT-slice [128, S]) with no
    transposes between layers.
  - k and v are ALSO produced token-major (v_tok[s, f]) "for free" by
    swapping matmul operands; attention probabilities are transposed with
    the PE transpose instruction.
  - heavy projection matmuls (QKV/O/FFN/vocab, attention ctx) run in bf16
    (weights host-cast to bf16, activations cast once per phase) with fp32
    PSUM accumulation; the residual stream, LayerNorms, attention scores
    (q.k), softmax, and all biases stay in full fp32.
"""

import sys

sys.path.insert(0, "/opt/trn_rl_repo")

import numpy as np

import concourse.bass as bass
import concourse.mybir as mybir
import concourse.tile as tile
from concourse import bacc
from concourse.bass import IndirectOffsetOnAxis
from concourse.masks import make_identity

P = 128
PAD_ID, SEP_ID = 0, 102
EPS = 1e-12
NEG = -1e9

F32 = mybir.dt.float32
F32R = mybir.dt.float32r
BF16 = mybir.dt.bfloat16
I32 = mybir.dt.int32

FULL_CFG = dict(S=512, H=768, NH=12, L=12, FF=3072, V=21128)
N_CORES = 8


def _r(ap):
    """Reinterpret an fp32 AP as float32r for fast matmul."""
    return ap.bitcast(F32R)


def _nslices(total, step=512):
    out = []
    s = 0
    while s < total:
        out.append((s, min(step, total - s)))
        s += step
    return out


def build_nc(cfg, debug=False):
    S, H, NH, L, FF, V = (
        cfg["S"], cfg["H"], cfg["NH"], cfg["L"], cfg["FF"], cfg["V"],
    )
    DH = 64
    assert H % P == 0 and S % P == 0 and FF % P == 0
    assert H // NH == DH
    KF = H // P          # feature subtiles (6)
    SC = S // P          # token chunks (4)
    KFF = FF // P        # ffn subtiles (24)
    HPG = P // DH        # heads per 128-partition group (2)

    nc = bacc.Bacc("TRN2", target_bir_lowering=False, debug=debug)

    # ---- I/O ------------------------------------------------------------
    ids_d = nc.declare_dram_parameter("input_ids", [S], I32, False)
    wemb_d = nc.declare_dram_parameter("word_emb", [V, H], F32, False)
    # pos_emb + type_emb[0] folded on the host
    pt_d = nc.declare_dram_parameter("pos_type_emb", [S, H], F32, False)
    embg_d = nc.declare_dram_parameter("emb_ln_g", [H], F32, False)
    embb_d = nc.declare_dram_parameter("emb_ln_b", [H], F32, False)
    aw_d = nc.declare_dram_parameter("attn_w", [L, 4, H, H], BF16, False)
    ab_d = nc.declare_dram_parameter("attn_b", [L, 4, H], F32, False)
    l1g_d = nc.declare_dram_parameter("ln1_g", [L, H], F32, False)
    l1b_d = nc.declare_dram_parameter("ln1_b", [L, H], F32, False)
    w1_d = nc.declare_dram_parameter("ffn_w1", [L, H, FF], BF16, False)
    b1_d = nc.declare_dram_parameter("ffn_b1", [L, FF], F32, False)
    w2_d = nc.declare_dram_parameter("ffn_w2", [L, FF, H], BF16, False)
    b2_d = nc.declare_dram_parameter("ffn_b2", [L, H], F32, False)
    l2g_d = nc.declare_dram_parameter("ln2_g", [L, H], F32, False)
    l2b_d = nc.declare_dram_parameter("ln2_b", [L, H], F32, False)
    clsw_d = nc.declare_dram_parameter("cls_w", [H, V], BF16, False)
    clsb_d = nc.declare_dram_parameter("cls_b", [V], F32, False)
    out_d = nc.declare_dram_parameter("out", [S, V], F32, True)

    with tile.TileContext(nc) as tc:
        _build_body(
            nc, tc,
            dict(S=S, H=H, NH=NH, L=L, FF=FF, V=V, DH=DH, KF=KF, SC=SC,
                 KFF=KFF, HPG=HPG),
            dict(ids=ids_d, wemb=wemb_d, pt=pt_d, embg=embg_d, embb=embb_d,
                 aw=aw_d, ab=ab_d, l1g=l1g_d, l1b=l1b_d, w1=w1_d, b1=b1_d,
                 w2=w2_d, b2=b2_d, l2g=l2g_d, l2b=l2b_d, clsw=clsw_d,
                 clsb=clsb_d, out=out_d),
        )
    nc.compile()
    return nc


def _build_body(nc, tc, c, d):
    S, H, NH, L, FF, V = c["S"], c["H"], c["NH"], c["L"], c["FF"], c["V"]
    DH, KF, SC, KFF, HPG = c["DH"], c["KF"], c["SC"], c["KFF"], c["HPG"]
    AL = mybir.AluOpType
    AF = mybir.ActivationFunctionType
    AX = mybir.AxisListType

    import contextlib

    ctx = contextlib.ExitStack()
    with ctx:
        const = ctx.enter_context(tc.tile_pool(name="const", bufs=1))
        persist = ctx.enter_context(tc.tile_pool(name="persist", bufs=1))
        wpool = ctx.enter_context(tc.tile_pool(name="wpool", bufs=4))
        w2pool = ctx.enter_context(tc.tile_pool(name="w2pool", bufs=4))
        bpool = ctx.enter_context(tc.tile_pool(name="bpool", bufs=2))
        spool = ctx.enter_context(tc.tile_pool(name="spool", bufs=3))
        ppool = ctx.enter_context(tc.tile_pool(name="ppool", bufs=2))
        opool = ctx.enter_context(tc.tile_pool(name="opool", bufs=3))
        psum = ctx.enter_context(tc.tile_pool(name="psum", bufs=8, space="PSUM"))

        # ---- constants --------------------------------------------------
        ident = const.tile([P, P], F32, name="ident")
        make_identity(nc, ident)
        ident_bf = const.tile([P, P], BF16, name="ident_bf")
        make_identity(nc, ident_bf)
        ones_col = const.tile([P, 1], F32, name="ones_col")   # lhsT for column sums
        nc.gpsimd.memset(ones_col[:], 1.0)
        ones_col_bf = const.tile([P, 1], BF16, name="ones_col_bf")
        nc.gpsimd.memset(ones_col_bf[:], 1.0)
        ones_row = const.tile([1, P], F32, name="ones_row")   # lhsT for broadcasts
        nc.gpsimd.memset(ones_row[:], 1.0)
        eps_col = const.tile([P, 1], F32, name="eps_col")
        nc.gpsimd.memset(eps_col[:], EPS)

        # ---- persistent activations ------------------------------------
        hT = persist.tile([P, KF, S], F32, name="hT")
        hT_bf = persist.tile([P, KF, S], BF16, name="hT_bf")
        qT = persist.tile([P, KF, S], BF16, name="qT")
        kT = persist.tile([P, KF, S], BF16, name="kT")
        v_tok = persist.tile([P, SC, H], BF16, name="v_tok")
        ctxT = persist.tile([P, KF, S], BF16, name="ctxT")
        attn_bias = persist.tile([P, SC, S], BF16, name="attn_bias")

        # =================================================================
        # Mask / additive attention bias from input_ids
        # =================================================================
        ids_row_i = spool.tile([1, S], I32, name="ids_row_i")
        nc.sync.dma_start(out=ids_row_i[:], in_=d["ids"][None, :])
        ids_row = const.tile([1, S], F32, name="ids_row")
        nc.vector.tensor_copy(out=ids_row[:], in_=ids_row_i[:])

        ids_p_i = spool.tile([P, SC], I32, name="ids_p_i")
        nc.sync.dma_start(
            out=ids_p_i[:], in_=d["ids"].rearrange("(c p) -> p c", p=P)
        )
        ids_p = const.tile([P, SC], F32, name="ids_p")
        nc.vector.tensor_copy(out=ids_p[:], in_=ids_p_i[:])

        iota_j_i = spool.tile([1, S], I32, name="iota_j_i")
        nc.gpsimd.iota(iota_j_i[:], pattern=[[1, S]], base=0, channel_multiplier=0)
        iota_j = const.tile([1, S], F32, name="iota_j")
        nc.vector.tensor_copy(out=iota_j[:], in_=iota_j_i[:])

        iota_i_i = spool.tile([P, SC], I32, name="iota_i_i")
        nc.gpsimd.iota(iota_i_i[:], pattern=[[P, SC]], base=0, channel_multiplier=1)
        iota_i = const.tile([P, SC], F32, name="iota_i")
        nc.vector.tensor_copy(out=iota_i[:], in_=iota_i_i[:])

        pad_j = const.tile([1, S], F32, name="pad_j")
        nc.vector.tensor_scalar(pad_j[:], ids_row[:], float(PAD_ID), None, AL.not_equal)
        pad_i = const.tile([P, SC], F32, name="pad_i")
        nc.vector.tensor_scalar(pad_i[:], ids_p[:], float(PAD_ID), None, AL.not_equal)

        # first-SEP position -> qlen = pos + 1
        sep = spool.tile([1, S], F32, name="sep", tag="lrow", bufs=4)
        nc.vector.tensor_scalar(sep[:], ids_row[:], float(SEP_ID), None, AL.is_equal)
        tsel = spool.tile([1, S], F32, name="tsel", tag="lrow", bufs=4)
        nc.vector.tensor_scalar(tsel[:], iota_j[:], float(S), None, AL.subtract)
        nc.vector.tensor_tensor(tsel[:], tsel[:], sep[:], AL.mult)
        nc.vector.tensor_scalar(tsel[:], tsel[:], float(S), None, AL.add)
        qlen = const.tile([1, 1], F32, name="qlen")
        nc.vector.tensor_reduce(qlen[:], tsel[:], axis=AX.X, op=AL.min)
        nc.vector.tensor_scalar(qlen[:], qlen[:], 1.0, None, AL.add)

        # broadcast qlen to all partitions (K=1 matmul)
        ps_q = psum.tile([P, 512], F32, name="ps", tag="ps")
        nc.tensor.matmul(ps_q[:, :1], lhsT=ones_row[:], rhs=qlen[:], start=True, stop=True)
        qlen_b = const.tile([P, 1], F32, name="qlen_b")
        nc.vector.tensor_copy(out=qlen_b[:], in_=ps_q[:, :1])

        a_i = const.tile([P, SC], F32, name="a_i")
        nc.vector.tensor_scalar(a_i[:], iota_i[:], qlen_b[:, :1], None, AL.is_ge)
        a_j = spool.tile([1, S], F32, name="a_j", tag="lrow", bufs=4)
        nc.vector.tensor_scalar(a_j[:], iota_j[:], qlen[:, :1], None, AL.is_ge)

        # broadcast a_j and pad_j across partitions
        ps_a = psum.tile([P, 512], F32, name="ps", tag="ps")
        a_jb = const.tile([P, S], F32, name="a_jb")
        for s0, sl in _nslices(S):
            nc.tensor.matmul(ps_a[:, :sl], lhsT=ones_row[:], rhs=a_j[:, s0:s0 + sl],
                             start=True, stop=True)
            nc.vector.tensor_copy(out=a_jb[:, s0:s0 + sl], in_=ps_a[:, :sl])
        ps_p = psum.tile([P, 512], F32, name="ps", tag="ps")
        pad_jb = const.tile([P, S], F32, name="pad_jb")
        for s0, sl in _nslices(S):
            nc.tensor.matmul(ps_p[:, :sl], lhsT=ones_row[:], rhs=pad_j[:, s0:s0 + sl],
                             start=True, stop=True)
            nc.vector.tensor_copy(out=pad_jb[:, s0:s0 + sl], in_=ps_p[:, :sl])

        for sc in range(SC):
            # U_c[p, j] = 1.0 if j > sc*128 + p else 0.0
            u_c = spool.tile([P, S], F32, name="u_c", tag="mask_s", bufs=2)
            nc.gpsimd.memset(u_c[:], 1.0)
            nc.gpsimd.affine_select(
                out=u_c[:], in_=u_c[:], compare_op=AL.is_gt, fill=0.0,
                base=-(sc * P), channel_multiplier=-1, pattern=[[1, S]],
            )
            t1 = spool.tile([P, S], F32, name="t1", tag="mask_s", bufs=2)
            nc.vector.tensor_tensor(t1[:], a_jb[:], u_c[:], AL.mult)
            nc.vector.tensor_scalar(t1[:], t1[:], a_i[:, sc:sc + 1], None, AL.mult)
            nc.vector.tensor_scalar(t1[:], t1[:], -1.0, 1.0, AL.mult, AL.add)
            nc.vector.tensor_tensor(t1[:], t1[:], pad_jb[:], AL.mult)
            nc.vector.tensor_scalar(t1[:], t1[:], pad_i[:, sc:sc + 1], None, AL.mult)
            # masked entries get -80: exp(-80) is a normal f32 and the masked
            # leakage (~e^-65 relative) rounds to exactly 0 in bf16 probs.
            nc.vector.tensor_scalar(
                attn_bias[:, sc, :], t1[:], 80.0, -80.0, AL.mult, AL.add
            )
        # rows with pad_i == 0 must come out as the uniform 1/S distribution
        # (reference: -1e9 bias absorbs the scores); b_fix = (1 - pad_i)/S
        b_fix = const.tile([P, SC], F32, name="b_fix")
        nc.vector.tensor_scalar(
            b_fix[:], pad_i[:], -1.0 / S, 1.0 / S, AL.mult, AL.add
        )

        # =================================================================
        # Embedding: gather + pos/type + LN  -> hT (feature-major)
        # =================================================================
        embg_b = wpool.tile([P, H], F32, name="embg_b", tag="w")
        nc.sync.dma_start(out=embg_b[:], in_=d["embg"][None, :].to_broadcast([P, H]))
        embb_b = wpool.tile([P, H], F32, name="embb_b", tag="w")
        nc.sync.dma_start(out=embb_b[:], in_=d["embb"][None, :].to_broadcast([P, H]))

        for sc in range(SC):
            idx_c = spool.tile([P, 1], I32, name="idx_c", tag="idx")
            nc.sync.dma_start(out=idx_c[:], in_=d["ids"][sc * P:(sc + 1) * P, None])
            g_c = spool.tile([P, H], F32, name="g_c", tag="tokh", bufs=2)
            nc.gpsimd.indirect_dma_start(
                out=g_c[:], out_offset=None, in_=d["wemb"][:],
                in_offset=IndirectOffsetOnAxis(ap=idx_c[:, :1], axis=0),
            )
            pt_c = spool.tile([P, H], F32, name="pt_c", tag="tokh_pt", bufs=2)
            nc.sync.dma_start(out=pt_c[:], in_=d["pt"][sc * P:(sc + 1) * P, :])
            nc.vector.tensor_tensor(g_c[:], g_c[:], pt_c[:], AL.add)

            # token-major LN over the free dim
            s1 = spool.tile([P, 1], F32, name="s1", tag="stat")
            nc.vector.reduce_sum(s1[:], g_c[:], axis=AX.X)
            mu = spool.tile([P, 1], F32, name="mu", tag="stat")
            nc.vector.tensor_scalar(mu[:], s1[:], 1.0 / H, None, AL.mult)
            sq_c = spool.tile([P, H], F32, name="sq_c", tag="tokh_sq", bufs=2)
            s2 = spool.tile([P, 1], F32, name="s2", tag="stat")
            nc.scalar.activation(sq_c[:], g_c[:], AF.Square, accum_out=s2[:])
            var = spool.tile([P, 1], F32, name="var", tag="stat")
            nc.vector.tensor_scalar(var[:], s2[:], 1.0 / H, None, AL.mult)
            mu2 = spool.tile([P, 1], F32, name="mu2", tag="stat")
            nc.vector.tensor_tensor(mu2[:], mu[:], mu[:], AL.mult)
            nc.vector.tensor_tensor(var[:], var[:], mu2[:], AL.subtract)
            sd = spool.tile([P, 1], F32, name="sd", tag="stat")
            nc.scalar.activation(sd[:], var[:], AF.Sqrt, bias=eps_col[:, :1])
            rstd = spool.tile([P, 1], F32, name="rstd", tag="stat")
            nc.vector.reciprocal(rstd[:], sd[:])

            # overwrite the (now dead) squares buffer with xhat
            xhat = sq_c
            nc.vector.tensor_scalar(
                xhat[:], g_c[:], mu[:, :1], rstd[:, :1], AL.subtract, AL.mult
            )
            nc.vector.tensor_tensor(xhat[:], xhat[:], embg_b[:], AL.mult)
            nc.vector.tensor_tensor(xhat[:], xhat[:], embb_b[:], AL.add)

            # transpose to feature-major
            for kf in range(KF):
                ps_t = psum.tile([P, 512], F32, name="ps", tag="ps")
                nc.tensor.transpose(
                    ps_t[:, :P], xhat[:, kf * P:(kf + 1) * P], ident[:]
                )
                nc.scalar.activation(
                    hT[:, kf, sc * P:(sc + 1) * P], ps_t[:, :P], AF.Identity
                )

        # =================================================================
        # Transformer layers
        # =================================================================
        for l in range(L):
            _layer(nc, c, d, l, dict(
                hT=hT, hT_bf=hT_bf, qT=qT, kT=kT, v_tok=v_tok, ctxT=ctxT,
                attn_bias=attn_bias, pad_i=pad_i, b_fix=b_fix,
                ident=ident, ident_bf=ident_bf,
                ones_col=ones_col, ones_col_bf=ones_col_bf,
                ones_row=ones_row, eps_col=eps_col,
                wpool=wpool, w2pool=w2pool, bpool=bpool, spool=spool,
                ppool=ppool, psum=psum,
            ))

        # =================================================================
        # Vocab projection: out[s, v] = h[s] @ cls_w + cls_b  (token-major)
        # =================================================================
        clsw_r = d["clsw"].rearrange("(ko p) v -> p ko v", p=P)
        out_r = d["out"].rearrange("(c p) v -> p c v", p=P)

        # final activations in bf16 for the vocab matmul
        for kf in range(KF):
            nc.vector.tensor_copy(out=hT_bf[:, kf, :], in_=hT[:, kf, :])
        for v0, vl in _nslices(V, 512):
            cw = wpool.tile([P, KF, 512], BF16, name="cw", tag="w")
            nc.sync.dma_start(out=cw[:, :, :vl], in_=clsw_r[:, :, v0:v0 + vl])
            clsb_sl = spool.tile([1, 512], F32, name="clsb_sl", tag="clsb", bufs=2)
            nc.sync.dma_start(out=clsb_sl[:, :vl], in_=d["clsb"][None, v0:v0 + vl])
            # bias broadcast tile for this slice (via SBUF: a tensor_tensor
            # may read at most one PSUM operand)
            ps_b = psum.tile([P, 512], F32, name="ps_b", tag="ps")
            nc.tensor.matmul(
                ps_b[:, :vl], lhsT=ones_row[:], rhs=clsb_sl[:, :vl],
                start=True, stop=True,
            )
            bias_bc = opool.tile([P, 512], F32, name="bias_bc", tag="bias_bc", bufs=2)
            nc.scalar.activation(bias_bc[:, :vl], ps_b[:, :vl], AF.Identity)
            for sc in range(SC):
                ps_o = psum.tile([P, 512], F32, name="ps_o", tag="ps")
                for kf in range(KF):
                    nc.tensor.matmul(
                        ps_o[:, :vl],
                        lhsT=hT_bf[:, kf, sc * P:(sc + 1) * P],
                        rhs=cw[:, kf, :vl],
                        start=(kf == 0), stop=(kf == KF - 1),
                    )
                o_sb = opool.tile([P, 512], F32, name="o_sb", tag="o")
                nc.vector.tensor_tensor(
                    o_sb[:, :vl], ps_o[:, :vl], bias_bc[:, :vl], AL.add
                )
                nc.sync.dma_start(
                    out=out_r[:, sc, v0:v0 + vl], in_=o_sb[:, :vl]
                )


def _layer(nc, c, d, l, t):
    S, H, NH, FF = c["S"], c["H"], c["NH"], c["FF"]
    DH, KF, SC, KFF, HPG = c["DH"], c["KF"], c["SC"], c["KFF"], c["HPG"]
    AL = mybir.AluOpType
    AF = mybir.ActivationFunctionType
    AX = mybir.AxisListType

    hT, qT, kT, v_tok, ctxT = (
        t["hT"], t["qT"], t["kT"], t["v_tok"], t["ctxT"]
    )
    hT_bf = t["hT_bf"]
    attn_bias, ident, ident_bf = t["attn_bias"], t["ident"], t["ident_bf"]
    pad_i, b_fix = t["pad_i"], t["b_fix"]
    ones_col, ones_row = t["ones_col"], t["ones_row"]
    ones_col_bf = t["ones_col_bf"]
    eps_col = t["eps_col"]
    wpool, w2pool, bpool = t["wpool"], t["w2pool"], t["bpool"]
    spool, ppool, psum = t["spool"], t["ppool"], t["psum"]

    # ---- helpers --------------------------------------------------------
    def wslice(w2d_ap, n0, nl, name):
        """Stream a [H, n0:n0+nl] weight slice as [128, KF, nl] (f-major)."""
        w = wpool.tile([P, KF, 512], BF16, name=name, tag="w")
        nc.sync.dma_start(
            out=w[:, :, :nl],
            in_=w2d_ap.rearrange("(ko p) n -> p ko n", p=P)[:, :, n0:n0 + nl],
        )
        return w

    def bcol(src_ap, name, scale=None):
        b = bpool.tile([P, KF], F32, name=name, tag="bcol")
        nc.sync.dma_start(out=b[:], in_=src_ap.rearrange("(ko p) -> p ko", p=P))
        if scale is not None:
            nc.vector.tensor_scalar(b[:], b[:], scale, None, AL.mult)
        return b

    with nc.named_scope("qkv"):
        # ======================================================================
        # q/k feature-major, v token-major
        # ======================================================================
        for kf in range(KF):
            nc.vector.tensor_copy(out=hT_bf[:, kf, :], in_=hT[:, kf, :])
        bq = bcol(d["ab"][l, 0], "bq", scale=1.0 / float(np.sqrt(DH)))
        for n0, nl in _nslices(H, 512):
            wq = wslice(d["aw"][l, 0], n0, nl, "wq")
            for msub in range(nl // P):
                m = n0 // P + msub
                ps = psum.tile([P, 512], F32, name="ps_qk", tag="ps")
                for kf in range(KF):
                    nc.tensor.matmul(
                        ps[:, :S],
                        lhsT=wq[:, kf, msub * P:(msub + 1) * P], rhs=hT_bf[:, kf, :],
                        start=(kf == 0), stop=(kf == KF - 1),
                    )
                nc.scalar.activation(
                    qT[:, m, :], ps[:, :S], AF.Identity,
                    bias=bq[:, m:m + 1], scale=1.0 / float(np.sqrt(DH)),
                )

        bk = bcol(d["ab"][l, 1], "bk")
        for n0, nl in _nslices(H, 512):
            wk = wslice(d["aw"][l, 1], n0, nl, "wk")
            for msub in range(nl // P):
                m = n0 // P + msub
                ps = psum.tile([P, 512], F32, name="ps_qk", tag="ps")
                for kf in range(KF):
                    nc.tensor.matmul(
                        ps[:, :S],
                        lhsT=wk[:, kf, msub * P:(msub + 1) * P], rhs=hT_bf[:, kf, :],
                        start=(kf == 0), stop=(kf == KF - 1),
                    )
                nc.scalar.activation(
                    kT[:, m, :], ps[:, :S], AF.Identity, bias=bk[:, m:m + 1]
                )

        bv_row = bpool.tile([1, H], F32, name="bv_row", tag="brow")
        nc.sync.dma_start(out=bv_row[:], in_=d["ab"][l, 2][None, :])
        for n0, nl in _nslices(H, 512):
            wv = wslice(d["aw"][l, 2], n0, nl, "wv")
            for sc in range(SC):
                ps = psum.tile([P, 512], F32, name="ps_v", tag="ps")
                for kf in range(KF):
                    nc.tensor.matmul(
                        ps[:, :nl],
                        lhsT=hT_bf[:, kf, sc * P:(sc + 1) * P],
                        rhs=wv[:, kf, :nl],
                        start=(kf == 0), stop=False,
                    )
                nc.tensor.matmul(
                    ps[:, :nl], lhsT=ones_row[:], rhs=bv_row[:, n0:n0 + nl],
                    start=False, stop=True,
                )
                nc.scalar.activation(
                    v_tok[:, sc, n0:n0 + nl], ps[:, :nl], AF.Identity
                )

    with nc.named_scope("attn"):
        # ======================================================================
        # attention per head
        # ======================================================================
        ctx_psums = {}
        for h in range(NH):
            kf_h = h // HPG
            p0 = (h % HPG) * DH
            q_h = qT[p0:p0 + DH, kf_h, :]
            k_h = kT[p0:p0 + DH, kf_h, :]

            probs = [
                ppool.tile([P, S], BF16, name=f"probs{ic}", tag="probs", bufs=8)
                for ic in range(SC)
            ]
            for ic in range(SC):
                ps_s = psum.tile([P, 512], F32, name="ps_s", tag="ps")
                nc.tensor.matmul(
                    ps_s[:, :S], lhsT=q_h[:, ic * P:(ic + 1) * P], rhs=k_h,
                    start=True, stop=False,
                )
                # mask bias folded in on the PE: psum += I.T @ bias
                nc.tensor.matmul(
                    ps_s[:, :S], lhsT=ident_bf[:], rhs=attn_bias[:, ic, :],
                    start=False, stop=True,
                )
                e_sb = spool.tile([P, S], F32, name="e_sb", tag="row_s", bufs=6)
                rowsum = spool.tile([P, 1], F32, name="rowsum", tag="stat")
                nc.scalar.activation(
                    e_sb[:], ps_s[:, :S], AF.Exp, accum_out=rowsum[:]
                )
                recip = spool.tile([P, 1], F32, name="recip", tag="stat")
                nc.vector.reciprocal(recip[:], rowsum[:])
                a_fix = spool.tile([P, 1], F32, name="a_fix", tag="stat")
                nc.vector.tensor_tensor(
                    a_fix[:], recip[:], pad_i[:, ic:ic + 1], AL.mult
                )
                nc.vector.tensor_scalar(
                    probs[ic][:], e_sb[:], a_fix[:, :1], b_fix[:, ic:ic + 1],
                    AL.mult, AL.add,
                )

            # transpose probs -> probsT (feature j on partitions)
            probsT = [
                ppool.tile([P, S], BF16, name=f"probsT{jc}", tag="probsT", bufs=8)
                for jc in range(SC)
            ]
            for jc in range(SC):
                ps_t = psum.tile([P, 512], BF16, name="ps_t", tag="ps")
                for ic in range(SC):
                    nc.tensor.transpose(
                        ps_t[:, ic * P:(ic + 1) * P],
                        probs[ic][:, jc * P:(jc + 1) * P], ident_bf[:],
                    )
                if jc % 2 == 0:
                    nc.scalar.activation(probsT[jc][:], ps_t[:, :S], AF.Identity)
                else:
                    nc.vector.tensor_copy(out=probsT[jc][:], in_=ps_t[:, :S])

            # ctx feature-major: a pair of heads shares one psum bank
            if h % HPG == 0:
                ps_c = psum.tile([P, 512], F32, name="ps_c", tag="ps")
                ctx_psums[kf_h] = ps_c
            ps_c = ctx_psums[kf_h]
            for jc in range(SC):
                nc.tensor.matmul(
                    ps_c[p0:p0 + DH, :S],
                    lhsT=v_tok[:, jc, h * DH:(h + 1) * DH],
                    rhs=probsT[jc][:],
                    start=(jc == 0), stop=(jc == SC - 1),
                    tile_position=(0, p0) if p0 else None,
                )
            if h % HPG == HPG - 1:
                nc.scalar.activation(ctxT[:, kf_h, :], ps_c[:, :S], AF.Identity)
                del ctx_psums[kf_h]

    with nc.named_scope("oproj"):
        # ======================================================================
        # attention out projection + residual + LN1
        # ======================================================================
        bo = bcol(d["ab"][l, 3], "bo")
        for n0, nl in _nslices(H, 512):
            wo = wslice(d["aw"][l, 3], n0, nl, "wo")
            for msub in range(nl // P):
                m = n0 // P + msub
                ps = psum.tile([P, 512], F32, name="ps_o", tag="ps")
                for kf in range(KF):
                    nc.tensor.matmul(
                        ps[:, :S],
                        lhsT=wo[:, kf, msub * P:(msub + 1) * P], rhs=ctxT[:, kf, :],
                        start=(kf == 0), stop=(kf == KF - 1),
                    )
                a_sb = spool.tile([P, S], F32, name="a_sb", tag="row_s", bufs=6)
                nc.scalar.activation(a_sb[:], ps[:, :S], AF.Identity, bias=bo[:, m:m + 1])
                nc.vector.tensor_tensor(hT[:, m, :], hT[:, m, :], a_sb[:], AL.add)

    _ln_feature_major(nc, c, hT, d["l1g"][l], d["l1b"][l], t, "ln1")

    with nc.named_scope("ffn"):
        # ======================================================================
        # FFN (blocked over FF so u never lives whole)
        # ======================================================================
        for kf in range(KF):
            nc.vector.tensor_copy(out=hT_bf[:, kf, :], in_=hT[:, kf, :])
        b1 = bpool.tile([P, KFF], F32, name="b1", tag="b1col")
        nc.sync.dma_start(out=b1[:], in_=d["b1"][l].rearrange("(ko p) -> p ko", p=P))
        b2 = bcol(d["b2"][l], "b2")
        w1_r = d["w1"][l].rearrange("(ko p) n -> p ko n", p=P)
        w2_r = d["w2"][l].rearrange("(ko p) n -> p ko n", p=P)
        ps_d = [
            psum.tile([P, 512], F32, name=f"ps_d{m}", tag="ps") for m in range(KF)
        ]
        for b0, bl in _nslices(FF, 512):
            w1s = wpool.tile([P, KF, 512], BF16, name="w1s", tag="w")
            nc.sync.dma_start(out=w1s[:, :, :bl], in_=w1_r[:, :, b0:b0 + bl])
            ublk = spool.tile([P, 4, S], BF16, name="ublk", tag="ublk", bufs=2)
            for j in range(bl // P):
                kff = b0 // P + j
                ps_u = psum.tile([P, 512], F32, name="ps_u", tag="ps")
                for kf in range(KF):
                    nc.tensor.matmul(
                        ps_u[:, :S],
                        lhsT=w1s[:, kf, j * P:(j + 1) * P], rhs=hT_bf[:, kf, :],
                        start=(kf == 0), stop=(kf == KF - 1),
                    )
                nc.scalar.activation(
                    ublk[:, j, :], ps_u[:, :S], AF.Gelu, bias=b1[:, kff:kff + 1]
                )
            for j in range(bl // P):
                kff = b0 // P + j
                w2c = w2pool.tile([P, H], BF16, name="w2c", tag="w2")
                nc.sync.dma_start(out=w2c[:], in_=w2_r[:, kff, :])
                for m in range(KF):
                    nc.tensor.matmul(
                        ps_d[m][:, :S],
                        lhsT=w2c[:, m * P:(m + 1) * P], rhs=ublk[:, j, :],
                        start=(kff == 0), stop=(kff == KFF - 1),
                    )
        for m in range(KF):
            d_sb = spool.tile([P, S], F32, name="d_sb", tag="row_s", bufs=6)
            nc.scalar.activation(d_sb[:], ps_d[m][:, :S], AF.Identity, bias=b2[:, m:m + 1])
            nc.vector.tensor_tensor(hT[:, m, :], hT[:, m, :], d_sb[:], AL.add)

    _ln_feature_major(nc, c, hT, d["l2g"][l], d["l2b"][l], t, "ln2")


def _ln_feature_major(nc, c, hT, g_dram, b_dram, t, name):
    """LayerNorm over the feature (partition) dim of feature-major hT, in place."""
    import contextlib
    _sc = contextlib.ExitStack()
    _sc.enter_context(nc.named_scope("ln"))
    S, H, KF = c["S"], c["H"], c["KF"]
    AL = mybir.AluOpType
    AF = mybir.ActivationFunctionType
    ones_col, ones_row = t["ones_col"], t["ones_row"]
    eps_col = t["eps_col"]
    spool, bpool, psum = t["spool"], t["bpool"], t["psum"]

    hT_bf = t["hT_bf"]
    ones_col_bf = t["ones_col_bf"]
    g_sb = bpool.tile([P, KF], mybir.dt.float32, name=f"{name}_g", tag="bcol")
    nc.sync.dma_start(out=g_sb[:], in_=g_dram.rearrange("(ko p) -> p ko", p=P))
    b_sb = bpool.tile([P, KF], mybir.dt.float32, name=f"{name}_b", tag="bcol")
    nc.sync.dma_start(out=b_sb[:], in_=b_dram.rearrange("(ko p) -> p ko", p=P))

    # stats from the bf16 copy: S1 = sum_f h, S2 = sum_f h^2. The per-element
    # bf16 rounding noise averages out over H; mean/var error is O(1e-4).
    for kf in range(KF):
        nc.vector.tensor_copy(out=hT_bf[:, kf, :], in_=hT[:, kf, :])
    ps_s1 = psum.tile([P, 512], F32, name=f"{name}_s1", tag="ps")
    ps_s2 = psum.tile([P, 512], F32, name=f"{name}_s2", tag="ps")
    for kf in range(KF):
        nc.tensor.matmul(
            ps_s1[:1, :S], lhsT=ones_col_bf[:, :1], rhs=hT_bf[:, kf, :],
            start=(kf == 0), stop=(kf == KF - 1),
        )
        sq = spool.tile([P, S], BF16, name=f"{name}_sq", tag="row_sbf")
        nc.scalar.activation(sq[:], hT_bf[:, kf, :], AF.Square)
        nc.tensor.matmul(
            ps_s2[:1, :S], lhsT=ones_col_bf[:, :1], rhs=sq[:],
            start=(kf == 0), stop=(kf == KF - 1),
        )

    mu = spool.tile([1, S], F32, name=f"{name}_mu", tag="lrow", bufs=4)
    nc.vector.tensor_scalar(mu[:], ps_s1[:1, :S], 1.0 / H, None, AL.mult)
    e2 = spool.tile([1, S], F32, name=f"{name}_e2", tag="lrow", bufs=4)
    nc.vector.tensor_scalar(e2[:], ps_s2[:1, :S], 1.0 / H, None, AL.mult)
    var = spool.tile([1, S], F32, name=f"{name}_var", tag="lrow", bufs=4)
    nc.vector.tensor_tensor(var[:], mu[:], mu[:], AL.mult)
    nc.vector.tensor_tensor(var[:], e2[:], var[:], AL.subtract)
    sd = spool.tile([1, S], F32, name=f"{name}_sd", tag="lrow", bufs=4)
    nc.scalar.activation(sd[:], var[:], AF.Sqrt, bias=eps_col[:1, :1])
    rstd = spool.tile([1, S], F32, name=f"{name}_rstd", tag="lrow", bufs=4)
    nc.vector.reciprocal(rstd[:], sd[:])
    # negated mu*rstd so the LN apply can use commutative ops with the
    # PSUM operand in slot 0 (a tensor_tensor may only read PSUM via in0)
    mrs = spool.tile([1, S], F32, name=f"{name}_mrs", tag="lrow", bufs=4)
    nc.vector.tensor_tensor(mrs[:], mu[:], rstd[:], AL.mult)
    nc.vector.tensor_scalar(mrs[:], mrs[:], -1.0, None, AL.mult)

    # broadcast rstd / mu*rstd across partitions (full fp32 matmuls)
    ps_r = psum.tile([P, 512], F32, name=f"{name}_br", tag="ps")
    ps_m = psum.tile([P, 512], F32, name=f"{name}_bm", tag="ps")
    for s0, sl in _nslices(S):
        nc.tensor.matmul(ps_r[:, s0:s0 + sl],
                         lhsT=ones_row[:], rhs=rstd[:, s0:s0 + sl],
                         start=(s0 == 0), stop=(s0 + sl >= S))
        nc.tensor.matmul(ps_m[:, s0:s0 + sl],
                         lhsT=ones_row[:], rhs=mrs[:, s0:s0 + sl],
                         start=(s0 == 0), stop=(s0 + sl >= S))

    for kf in range(KF):
        tt = spool.tile([P, S], F32, name=f"{name}_t", tag="row_s", bufs=6)
        nc.vector.tensor_tensor(tt[:], ps_r[:, :S], hT[:, kf, :], AL.mult)
        nc.vector.tensor_tensor(tt[:], ps_m[:, :S], tt[:], AL.add)
        nc.vector.tensor_scalar(
            hT[:, kf, :], tt[:], g_sb[:, kf:kf + 1], b_sb[:, kf:kf + 1],
            AL.mult, AL.add,
        )
    _sc.close()


# =========================================================================
# Host entry point
# =========================================================================

_NC_CACHE = {}


def _get_nc():
    key = "full"
    if key not in _NC_CACHE:
        _NC_CACHE[key] = build_nc(FULL_CFG)
    return _NC_CACHE[key]


def _prep_in_maps(inputs):
    import ml_dtypes

    cfg = FULL_CFG
    B = N_CORES
    ids = np.asarray(inputs["input_ids"], dtype=np.int32)
    assert ids.shape == (B, cfg["S"])

    pos_type = (
        np.asarray(inputs["pos_emb"], np.float32)
        + np.asarray(inputs["type_emb"], np.float32)[0][None, :]
    )
    bf = lambda k: np.ascontiguousarray(
        np.asarray(inputs[k], np.float32).astype(ml_dtypes.bfloat16)
    )

    shared = {
        "word_emb": np.ascontiguousarray(inputs["word_emb"], np.float32),
        "pos_type_emb": np.ascontiguousarray(pos_type, np.float32),
        "emb_ln_g": np.ascontiguousarray(inputs["emb_ln_g"], np.float32),
        "emb_ln_b": np.ascontiguousarray(inputs["emb_ln_b"], np.float32),
        "attn_w": bf("attn_w"),
        "attn_b": np.ascontiguousarray(inputs["attn_b"], np.float32),
        "ln1_g": np.ascontiguousarray(inputs["ln1_g"], np.float32),
        "ln1_b": np.ascontiguousarray(inputs["ln1_b"], np.float32),
        "ffn_w1": bf("ffn_w1"),
        "ffn_b1": np.ascontiguousarray(inputs["ffn_b1"], np.float32),
        "ffn_w2": bf("ffn_w2"),
        "ffn_b2": np.ascontiguousarray(inputs["ffn_b2"], np.float32),
        "ln2_g": np.ascontiguousarray(inputs["ln2_g"], np.float32),
        "ln2_b": np.ascontiguousarray(inputs["ln2_b"], np.float32),
        "cls_w": bf("cls_w"),
        "cls_b": np.ascontiguousarray(inputs["cls_b"], np.float32),
    }
    in_maps = [
        {"input_ids": np.ascontiguousarray(ids[i]), **shared} for i in range(B)
    ]
    return in_maps


def _run(inputs, trace=False, **kw):
    from concourse.bass_utils import run_bass_kernel_spmd

    in_maps = _prep_in_maps(inputs)
    nc = _get_nc()
    res = run_bass_kernel_spmd(nc, in_maps, list(range(N_CORES)), trace=trace, **kw)
    out = np.stack(
        [res.results[i]["out"] for i in range(N_CORES)], axis=0
    ).astype(np.float32)
    return out, res


def kernel(**inputs):
    out, _ = _run(inputs, trace=False)
    return out


def run_traced(**inputs):
    return _run(inputs, trace=True)

